# revision 3
# baseline (speedup 1.0000x reference)
"""Trainium2 Bass kernel for nn_CI3addFrom01 (segment_reduce), v3.

Reference computation:
    out[b] = sum_m softmax(preweight)[m] * max_k min_j x[b, idx[m,k,j]]
with M = 40704 antichains over DIM = 32.

v2 key idea: for E > 0, E*max(a,b) = max(E*a, E*b) and likewise for min,
so the softmax weight of every single-chain antichain is folded into the
one-hot matmul table on the host ("pre-scaled" columns).  The weighted
sum over the R1 class (single-group antichains: min3 of a scaled column)
then rides for free on the accum_out of the final chain op (STT with
op1=min sums its outputs).  R2 (all-singleton antichains: max3) reuses
the same scaled columns and keeps an explicit weighted-sum pass with
RATIO weights E2/E1.  The R3a/R3b grids (singleton-vs-pair and
pair-vs-pair) keep the baseline broadcast-grid + weighted-sum structure
since their operands are shared across cells with different weights.

Table layout per core (NTAB2 = 936 cols, single PSUM round):
    [R1a'(32) R1b'(62) R1c'(620) | X(128) P(62) | S(32)]
    ' = E1-scaled; X/P/S unscaled (grid operands).
    j-gathers: j0 all 936; j1 [0:904]; j2 [0:714].
xcall2 = [R2v(682) | R3a(1984) | R3b(2560)] aligned with the E table
(pwe2): R2 ratio weights exp(pw2-pw1), R3a/R3b absolute exp weights.
Host sums the three per-core partial outputs and divides by Z.
"""

import itertools
import math

import numpy as np

DIM = 32
B = 512
NCORES = 8
NPAIR_C = 62
NTRIP_C = 620
NM_C = 4
XPAD = 32

PAIRS = list(itertools.combinations(range(DIM), 2))
TRIPS = list(itertools.combinations(range(DIM), 3))
TRIPIDX = {t: i for i, t in enumerate(TRIPS)}

N_R1 = 32 + NPAIR_C + NTRIP_C          # 714
N_R2 = NPAIR_C + NTRIP_C               # 682
NX = NM_C * XPAD                       # 128
# table column layout: [R1' | R2' | X P | S]
T_R1 = (0, N_R1)
T_R2 = (N_R1, N_R1 + N_R2)             # 714:1396
T_X = (T_R2[1], T_R2[1] + NX)          # 1396:1524
T_P = (T_X[1], T_X[1] + NPAIR_C)       # 1524:1586
T_S = (T_P[1], T_P[1] + 32)            # 1586:1618
NTAB2 = T_S[1]                         # 1618
NEG = -1e30

R3B_GRIDS = [
    ("rAB", 16, 16, 0, 16, False),
    ("rA", 8, 8, 0, 8, False),
    ("tA1", 8, 8, 0, 0, True),
    ("tA2", 8, 8, 8, 8, True),
    ("rB", 8, 8, 16, 24, False),
    ("tB1", 8, 8, 16, 16, True),
    ("tB2", 8, 8, 24, 24, True),
]

# xcall2 / pwe2 layout
SEG = {}
_off = 0
for _name, _ln in (
    [("R3a", 32 * NPAIR_C)]
    + [(n, NM_C * u * v) for n, u, v, _, _, _ in R3B_GRIDS]
):
    SEG[_name] = (_off, _off + _ln)
    _off += _ln
LPACK = _off                            # 4544
WSPLIT = 32 * NPAIR_C                   # 1984 (R3a | R3b)

# --- knobs ---
R3A_MAT = True      # materialize R3a singleton operand (rep-each) on ACT
R3B_MAT = True      # materialize R3b u-operands on ACT
_WP_BUFS = 3
_EP_BUFS = 2


def _others(m):
    return [x for x in range(DIM) if x != m]


_HOST_CACHE = {}


def _table_sets(core):
    """(set, scale_kind) per column; scale_kind: ('r1a',i)/('r1b',p)/
    ('r1c',q)/None."""
    sets = []
    for i in range(DIM):
        sets.append(((i, i, i), ("r1a", i)))
    for p in range(NPAIR_C * core, NPAIR_C * (core + 1)):
        i, j = PAIRS[p]
        sets.append(((i, j, j), ("r1b", p)))
    for q in range(NTRIP_C * core, NTRIP_C * (core + 1)):
        sets.append((TRIPS[q], ("r1c", q)))
    for p in range(NPAIR_C * core, NPAIR_C * (core + 1)):
        i, j = PAIRS[p]
        sets.append(((i, j, j), ("r2b", p)))
    for q in range(NTRIP_C * core, NTRIP_C * (core + 1)):
        sets.append((TRIPS[q], ("r2c", q)))
    for m in range(NM_C * core, NM_C * (core + 1)):
        for x in _others(m):
            sets.append(((min(m, x), max(m, x), max(m, x)), None))
        sets.append(((0, 0, 0), None))  # pad slot 31
    for p in range(NPAIR_C * core, NPAIR_C * (core + 1)):
        i, j = PAIRS[p]
        sets.append(((i, j, j), None))
    for i in range(DIM):
        sets.append(((i, i, i), None))
    return sets


def _onehots(core, pw):
    pw = np.asarray(pw, dtype=np.float64).reshape(-1)
    sets = _table_sets(core)
    assert len(sets) == NTAB2
    oh = np.zeros((3, DIM, NTAB2), dtype=np.float64)
    for c, (st, sk) in enumerate(sets):
        if sk is None:
            sc = 1.0
        elif sk[0] == "r1a":
            sc = math.exp(pw[sk[1]]) if core == 0 else 0.0
        elif sk[0] == "r1b":
            sc = math.exp(pw[32 + 2 * sk[1]])
        elif sk[0] == "r2b":
            sc = math.exp(pw[32 + 2 * sk[1] + 1])
        elif sk[0] == "r2c":
            sc = math.exp(pw[1024 + 8 * sk[1] + 1])
        else:
            sc = math.exp(pw[1024 + 8 * sk[1]])
        for j in range(3):
            oh[j, st[j], c] = sc
    return oh


def _widx_r3a(core):
    if ("r3a", core) in _HOST_CACHE:
        return _HOST_CACHE[("r3a", core)]
    g = np.full((32, NPAIR_C), -1, dtype=np.int64)
    for a in range(32):
        for pl, p in enumerate(range(NPAIR_C * core, NPAIR_C * (core + 1))):
            b, c = PAIRS[p]
            if a == b or a == c:
                continue
            tri = tuple(sorted((a, b, c)))
            g[a, pl] = 1024 + 8 * TRIPIDX[tri] + 2 + tri.index(a)
    _HOST_CACHE[("r3a", core)] = g
    return g


def _r3b_widx_id(m, ou, ov):
    tri = tuple(sorted((m, ou, ov)))
    return 1024 + 8 * TRIPIDX[tri] + (7, 5, 6)[tri.index(m)]


def _widx_r3b(core):
    if ("r3b", core) in _HOST_CACHE:
        return _HOST_CACHE[("r3b", core)]
    grids = {}
    for name, ud, vd, uo, vo, halved in R3B_GRIDS:
        g = np.full((NM_C, ud, vd), -1, dtype=np.int64)
        for ml, m in enumerate(range(NM_C * core, NM_C * (core + 1))):
            ot = _others(m)
            for u in range(ud):
                for v in range(vd):
                    iu, iv = uo + u, vo + v
                    if iu >= 31 or iv >= 31:
                        continue
                    gu, gv = ot[iu], ot[iv]
                    if gu != gv:
                        g[ml, u, v] = _r3b_widx_id(m, gu, gv)
        grids[name] = (g, halved)
    _HOST_CACHE[("r3b", core)] = grids
    return grids


def _packed_pwe(core, pw):
    """pwe2: [R2 ratio weights | R3a | R3b] (already exp'd)."""
    pw = np.asarray(pw, dtype=np.float64).reshape(-1)
    out = np.full(LPACK, NEG, dtype=np.float64)
    items = [("R3a", _widx_r3a(core), False)]
    r3b = _widx_r3b(core)
    for name, _, _, _, _, _ in R3B_GRIDS:
        g, halved = r3b[name]
        items.append((name, g, halved))
    for name, grid, halved in items:
        flat = grid.reshape(-1)
        vals = np.full(flat.shape, NEG, dtype=np.float64)
        ok = flat >= 0
        vals[ok] = pw[flat[ok]]
        if halved:
            vals[ok] -= math.log(2.0)
        s, e = SEG[name]
        out[s:e] = vals
    return np.exp(out)


def _expected_idx():
    acs = [((i,),) for i in range(DIM)]
    for i, j in PAIRS:
        acs.append(((i, j),))
        acs.append(((i,), (j,)))
    for i, j, k in TRIPS:
        acs += [((i, j, k),), ((i,), (j,), (k,)), ((i,), (j, k)),
                ((j,), (i, k)), ((k,), (i, j)), ((i, j), (j, k)),
                ((i, k), (j, k)), ((i, j), (i, k))]
    idx = np.zeros((len(acs), 3, 3), dtype=np.int32)
    for m, ac in enumerate(acs):
        groups = [list(g) + [g[-1]] * (3 - len(g)) for g in ac]
        while len(groups) < 3:
            groups.append(groups[-1])
        idx[m] = np.array(groups, dtype=np.int32)
    return idx


_NC_CACHE = {}


def _build_nc(reps=1):
    import concourse.mybir as mybir
    from concourse import bacc
    from concourse.tile import TileContext

    f32 = mybir.dt.float32
    bf16 = mybir.dt.bfloat16
    Alu = mybir.AluOpType

    nc = bacc.Bacc(None, target_bir_lowering=False, debug=False)
    xT_d = nc.dram_tensor("xT", [DIM, B], bf16, kind="ExternalInput")
    oh_d = nc.dram_tensor("oh", [DIM, 3 * NTAB2], bf16, kind="ExternalInput")
    pwe_d = nc.dram_tensor("pwe", [1, LPACK], bf16, kind="ExternalInput")
    outd_d = nc.dram_tensor("outd", [B, 1], f32, kind="ExternalOutput")
    outa_d = nc.dram_tensor("outa", [B, 1], f32, kind="ExternalOutput")
    outp_d = nc.dram_tensor("outp", [B, 1], f32, kind="ExternalOutput")
    outq_d = nc.dram_tensor("outq", [B, 1], f32, kind="ExternalOutput")

    with TileContext(nc) as tc:
        with (
            tc.tile_pool(name="const", bufs=1) as cp,
            tc.tile_pool(name="ep", bufs=_EP_BUFS) as ep,
            tc.tile_pool(name="work", bufs=_WP_BUFS) as wp,
            tc.tile_pool(name="junkp", bufs=2) as jp,
            tc.tile_pool(name="pg", bufs=1, space="PSUM") as pg_pool,
        ):
            oh_t = cp.tile([DIM, 3 * NTAB2], bf16)
            xt_t = cp.tile([DIM, B], bf16)
            outsum_b = cp.tile([128, 16], f32)
            nc.vector.memset(outsum_b[:], 0.0)
            # E is constant across reps and batch tiles: one broadcast
            # DMA at kernel start (overlaps the first tile's matmuls)
            E = cp.tile([128, LPACK], bf16)
            nc.sync.dma_start(oh_t[:], oh_d[:])
            nc.sync.dma_start(xt_t[:], xT_d[:])
            nc.sync.dma_start(E[:], pwe_d[0:1, :].broadcast_to(
                [128, LPACK]))

            for _rep in range(reps):
                # per-rep accum targets (rotating bufs so reps pipeline);
                # chained into outsum_b below so no rep is dead code
                acc_b = wp.tile([128, 16], f32, tag="acc")
                outd_b = acc_b[:, 0:4]
                outa_b = acc_b[:, 4:8]
                outp_b = acc_b[:, 8:12]
                outq_b = acc_b[:, 12:16]
                for t in range(4):
                    # two matmul rounds: cols [0:1024], [1024:1618]
                    c012 = wp.tile([128, 3 * NTAB2], bf16, tag="c012")
                    jw = (NTAB2, T_S[0], T_X[0])  # j0/j1/j2 widths
                    for r0, r1 in ((0, 1024), (1024, NTAB2)):
                        G = pg_pool.tile([128, 3 * 1024], f32, tag="G")
                        for j in range(3):
                            for s in range(r0, min(jw[j], r1), 512):
                                e = min(s + 512, jw[j], r1)
                                nc.tensor.matmul(
                                    G[:, j * 1024 + s - r0:
                                      j * 1024 + e - r0],
                                    xt_t[:, t * 128:(t + 1) * 128],
                                    oh_t[:, j * NTAB2 + s: j * NTAB2 + e],
                                    start=True, stop=True)
                        w = r1 - r0
                        nc.scalar.copy(
                            c012[:].rearrange("p (j c) -> p j c",
                                              j=3)[:, :, r0:r1],
                            G[:].rearrange("p (j c) -> p j c", j=3,
                                           c=1024)[:, :, 0:w])
                    c0 = c012[:, 0:NTAB2]
                    c1 = c012[:, NTAB2:2 * NTAB2]
                    c2 = c012[:, 2 * NTAB2:3 * NTAB2]

                    # R1 chain: min3 over [0:714], STT accum -> outd
                    t1x = wp.tile([128, N_R1], bf16, tag="t1x")
                    nc.vector.tensor_tensor(t1x[:], c0[:, 0:N_R1],
                                            c1[:, 0:N_R1], Alu.min)
                    junk1 = jp.tile([128, N_R1], bf16, tag="junk1")
                    nc.vector.scalar_tensor_tensor(
                        junk1[:], t1x[:], 1.0, c2[:, 0:N_R1],
                        op0=Alu.mult, op1=Alu.min,
                        accum_out=outd_b[:, t:t + 1])

                    # XP: min(c0,c1) over [1396:1586]
                    xp_t = wp.tile([128, T_P[1] - T_X[0]], bf16, tag="xp")
                    nc.vector.tensor_tensor(xp_t[:], c0[:, T_X[0]:T_P[1]],
                                            c1[:, T_X[0]:T_P[1]], Alu.min)

                    xcall = wp.tile([128, LPACK], bf16, tag="xcall")
                    # R2 chain: max3 over scaled cols [714:1396], STT accum
                    t2 = wp.tile([128, N_R2], bf16, tag="t2")
                    nc.vector.tensor_tensor(t2[:], c0[:, T_R2[0]:T_R2[1]],
                                            c1[:, T_R2[0]:T_R2[1]], Alu.max)
                    junk2 = jp.tile([128, N_R2], bf16, tag="junk2")
                    nc.vector.scalar_tensor_tensor(
                        junk2[:], t2[:], 1.0, c2[:, T_R2[0]:T_R2[1]],
                        op0=Alu.mult, op1=Alu.max,
                        accum_out=outa_b[:, t:t + 1])

                    # R3a grid: max(S_a, P_p)
                    xS = c0[:, T_S[0]:T_S[1]]
                    xP = xp_t[:, T_P[0] - T_X[0]:T_P[1] - T_X[0]]
                    r3a = xcall[:, SEG["R3a"][0]:SEG["R3a"][1]].rearrange(
                        "p (a q) -> p a q", a=32)
                    pair_b = (xP[:].unsqueeze(1)
                              .broadcast_to([128, 32, NPAIR_C]))
                    if R3A_MAT:
                        r3aS = wp.tile([128, 32 * NPAIR_C], bf16, tag="r3aS")
                        nc.scalar.copy(
                            r3aS[:].rearrange("p (a q) -> p a q", a=32),
                            xS[:].unsqueeze(2)
                            .broadcast_to([128, 32, NPAIR_C]))
                        nc.vector.tensor_tensor(
                            r3a[:],
                            r3aS[:].rearrange("p (a q) -> p a q", a=32),
                            pair_b, Alu.max)
                    else:
                        nc.vector.tensor_tensor(
                            r3a[:],
                            xS[:].unsqueeze(2)
                            .broadcast_to([128, 32, NPAIR_C]),
                            pair_b, Alu.max)

                    junk = jp.tile([128, LPACK], bf16, tag="junk")
                    # wsum chunk A: R3a
                    nc.vector.scalar_tensor_tensor(
                        junk[:, 0:WSPLIT], xcall[:, 0:WSPLIT], 1.0,
                        E[:, 0:WSPLIT], op0=Alu.mult, op1=Alu.mult,
                        accum_out=outp_b[:, t:t + 1])

                    # R3b grids over X
                    xv = xp_t[:, 0:NX].rearrange(
                        "p (m t) -> p m t", m=NM_C)
                    if R3B_MAT:
                        r3bS = wp.tile(
                            [128, SEG["tB2"][1] - SEG["rAB"][0]], bf16,
                            tag="r3bS")
                    off0 = SEG["rAB"][0]
                    for name, ud, vd, uo, vo, _ in R3B_GRIDS:
                        s, e = SEG[name]
                        dst = xcall[:, s:e].rearrange(
                            "p (m u v) -> p m u v", m=NM_C, u=ud)
                        u_b = (xv[:, :, uo:uo + ud].unsqueeze(3)
                               .broadcast_to([128, NM_C, ud, vd]))
                        if R3B_MAT:
                            um = r3bS[:, s - off0:e - off0].rearrange(
                                "p (m u v) -> p m u v", m=NM_C, u=ud)
                            nc.scalar.copy(um, u_b)
                            u_b = um
                        nc.vector.tensor_tensor(
                            dst, u_b,
                            xv[:, :, vo:vo + vd].unsqueeze(2)
                            .broadcast_to([128, NM_C, ud, vd]),
                            Alu.max)

                    # wsum chunk B: R3b
                    nc.vector.scalar_tensor_tensor(
                        junk[:, WSPLIT:LPACK], xcall[:, WSPLIT:LPACK], 1.0,
                        E[:, WSPLIT:LPACK], op0=Alu.mult, op1=Alu.mult,
                        accum_out=outq_b[:, t:t + 1])

                nc.vector.tensor_tensor(outsum_b[:], outsum_b[:],
                                        acc_b[:], Alu.add)

            for t in range(4):
                nc.sync.dma_start(outd_d[t * 128:(t + 1) * 128, :],
                                  outsum_b[:, t:t + 1])
                nc.sync.dma_start(outa_d[t * 128:(t + 1) * 128, :],
                                  outsum_b[:, 4 + t:5 + t])
                nc.sync.dma_start(outp_d[t * 128:(t + 1) * 128, :],
                                  outsum_b[:, 8 + t:9 + t])
                nc.sync.dma_start(outq_d[t * 128:(t + 1) * 128, :],
                                  outsum_b[:, 12 + t:13 + t])
    nc.finalize()
    return nc


def make_in_maps(x, pw):
    import ml_dtypes

    bf = ml_dtypes.bfloat16
    xT = np.ascontiguousarray(np.asarray(x, np.float32).T.astype(bf))
    in_maps = []
    for core in range(NCORES):
        oh = _onehots(core, pw)
        in_maps.append({
            "xT": xT,
            "oh": np.ascontiguousarray(
                oh.transpose(1, 0, 2).reshape(DIM, 3 * NTAB2).astype(bf)),
            "pwe": _packed_pwe(core, pw).reshape(1, LPACK).astype(bf),
        })
    return in_maps


def kernel(x, preweight, idx):
    from concourse.bass_utils import run_bass_kernel_spmd

    x = np.ascontiguousarray(np.asarray(x, dtype=np.float32))
    pw = np.asarray(preweight, dtype=np.float32).reshape(-1)
    idx = np.asarray(idx)
    if not np.array_equal(idx, _expected_idx()):
        raise ValueError("idx does not match the expected antichain table")

    if "nc" not in _NC_CACHE:
        _NC_CACHE["nc"] = _build_nc()
    nc = _NC_CACHE["nc"]

    in_maps = make_in_maps(x, pw)
    res = run_bass_kernel_spmd(nc, in_maps, core_ids=list(range(NCORES)))
    total = np.zeros((B, 1), dtype=np.float64)
    for r in res.results:
        total += r["outd"].astype(np.float64)
        total += r["outa"].astype(np.float64)
        total += r["outp"].astype(np.float64)
        total += r["outq"].astype(np.float64)
    z = float(np.sum(np.exp(pw.astype(np.float64))))
    return (total / z).astype(np.float32)


if __name__ == "__main__":
    import reference
    inputs = {k: np.asarray(v) for k, v in reference.setup_inputs().items()}
    expected = np.asarray(reference.reference(**inputs))
    actual = kernel(**inputs)
    rel = np.abs(actual - expected).max() / np.abs(expected).max()
    print(f"kernel2 relative error: {rel:.3e}")
